# revision 27
# baseline (speedup 1.0000x reference)
# Trainium2 Bass kernel for nn_Decoder (LSTM decoder w/ Luong attention + vocab projection)
#
# Sharding: 8 cores = 2 batch-groups x 4 vocab-slices.
#   Each core runs the full recurrence for its 32-batch group (replicated across
#   the 4 vocab-slice cores of that group) and computes logits for its 8000-wide
#   vocab slice, streamed out to HBM during the recurrence.
#
# Structure vs the fp32 baseline:
#   * z lives in FOUR per-u-quarter PSUM tiles [32,256] (dst partition 0), so
#     the h/ctx chunks run as fp32r at full rate (1c/row at N=256) -- fp32r
#     cannot write PSUM partition offsets (s3d3_mm_valid_dst_partition).
#     Gates tanh becomes 4 ACTs [32,256] -> STATE rows 32q+b.
#   * attn is folded into the recurrence: for t>=1
#       z(t) = x@Wkx + ht(t-1)@(0.5(W_r + W_attn[:U]@Wk_a)) + ctx(t-1)@(W_attn[U:]@Wk_a)
#     so the serial path ends ctx -> ctxK -> z_ctx (attn-proj runs off-path,
#     feeding only the fc lhsT). t=0 uses original W_r (attn(0)=0).
#   * the softmax block-diag mask is a rank-5 matmul accumulated into the
#     scores PSUM (valid lanes get -B+B=0 exactly), so exp reads PS directly.
#   * wfc is bf16 (halved preamble DMA); fc lhsT is a bf16 mirror of attn_fcT.
#   * ctx normalization on DVE (recip + tensor_scalar mul), off the ACT queue.
# Numerics: state kept doubled (ct=2c, ht=2h); single ACT table (tanh/exp);
#   sigmoid(x) = (1+tanh(x/2))/2 with 0.5 compensations folded into weights.
# Layouts: as baseline -- STATE partition (32q+b); h K-form HBT [128, 2*32] via
#   one DVE StreamTranspose (stperm); ctx K-form natural via 2 PE transposes.
import numpy as np
from contextlib import ExitStack

import concourse.bass as bass
import concourse.bacc as bacc
import concourse.mybir as mybir
import concourse.tile as tile
from concourse import bass_utils

B, T, S, E, U, V = 64, 30, 31, 128, 256, 32000
NB, NV = 2, 4                  # batch groups x vocab slices = 8 cores
BL, VL = B // NB, V // NV      # 32 local batch, 8000 local vocab
SP = 32                        # padded source length
NCH = (BL * SP) // 128         # 8 block-diag / (b,s) chunks
UC = U + 4                     # context matmul width (col U = softmax denom)
FCB = 500                      # fc psum bank width (<=512)
NFCB = VL // FCB               # 16
MROWS = T * BL                 # 960 fc rows, (t,b) order
AFT = (T + 1) * BL             # attn_fcT cols (slice t+1 = attn(t))
BIGB = 300.0                   # mask bias magnitude
F32 = mybir.dt.float32
BF16 = mybir.dt.bfloat16
AF = mybir.ActivationFunctionType
ALU = mybir.AluOpType


def _r(ap):
    """fp32r view (PE rounds operands to ~12-bit mantissa, full rate N>=256)."""
    return ap.bitcast(mybir.dt.float32r)


def _ap(t, offset, dims):
    """Custom access pattern on a tile (flat element space)."""
    return bass.AP(t.tensor, t.offset + offset, dims)


def build_program(has_blstm: bool, repeat: int = 1, dbg: bool = False, zr: bool = True,
                  do_fc: bool = True, do_attn: bool = True, amp: str = ''):
    nc = bacc.Bacc("TRN2", target_bir_lowering=False, debug=False,
                   enable_asserts=False, num_devices=NB * NV)
    f = F32
    mm = lambda out, lhsT, rhs, **kw: nc.tensor.matmul(
        out, _r(lhsT), _r(rhs), skip_group_check=True, **kw)
    mmz = mm if zr else (lambda out, lhsT, rhs, **kw: nc.tensor.matmul(
        out, lhsT, rhs, skip_group_check=True, **kw))
    mmf = lambda out, lhsT, rhs, **kw: nc.tensor.matmul(
        out, lhsT, rhs, skip_group_check=True, **kw)
    # ---- per-core external I/O ----
    xT = nc.dram_tensor("xT", [E, T * BL], BF16, kind="ExternalInput").ap()
    # wbig chunks: 0,1 = Wr_orig (t=0); 2,3 = Wr_eff; 4,5 = Wca
    wbig = nc.dram_tensor("wbig", [6, 128, 4 * 256], f, kind="ExternalInput").ap()
    wbx = nc.dram_tensor("wbx", [128, 4 * 256], BF16, kind="ExternalInput").ap()
    wattn = nc.dram_tensor("wattn", [128, 4 * 2 * 128], f, kind="ExternalInput").ap()
    wmk = nc.dram_tensor("wmk", [128, 2 * 2 * 128], f, kind="ExternalInput").ap()
    wfc = nc.dram_tensor("wfc", [2, 128, VL], BF16, kind="ExternalInput").ap()
    memp = nc.dram_tensor("memp", [128, NCH * UC], f, kind="ExternalInput").ap()
    mempT = nc.dram_tensor("mempT", [128, 2 * BL * SP], f, kind="ExternalInput").ap()
    maskB = nc.dram_tensor("maskB", [128, NCH * BL], f, kind="ExternalInput").ap()
    hk0 = nc.dram_tensor("hk0", [128, 64], f, kind="ExternalInput").ap()
    c0 = nc.dram_tensor("c0", [128, 64], f, kind="ExternalInput").ap()
    eye = nc.dram_tensor("eye", [32, 32], f, kind="ExternalInput").ap()
    if has_blstm:
        blstm = nc.dram_tensor("blstm", [1, 4 * 256], f, kind="ExternalInput").ap()
        onesd = nc.dram_tensor("onesd", [1, BL], f, kind="ExternalInput").ap()
    out = nc.dram_tensor("out", [MROWS, VL], BF16, kind="ExternalOutput").ap()
    if dbg:
        dbg_t = nc.dram_tensor("dbg", [8, 128, 256], F32, kind="ExternalOutput").ap()

    with tile.TileContext(nc) as tc, ExitStack() as ctx:
        const = ctx.enter_context(tc.tile_pool(name="const", bufs=1))
        work = ctx.enter_context(tc.tile_pool(name="work", bufs=2))
        ps_z = ctx.enter_context(tc.tile_pool(name="ps_z", bufs=1, space="PSUM"))
        ps_sc = ctx.enter_context(tc.tile_pool(name="ps_sc", bufs=1, space="PSUM"))
        ps_ca = ctx.enter_context(tc.tile_pool(name="ps_ca", bufs=1, space="PSUM"))
        ps_fc = ctx.enter_context(tc.tile_pool(name="ps_fc", bufs=4, space="PSUM"))

        # ---- load constants into SBUF (wfc last: first consumed at t=4) ----
        xTs = const.tile([E, T * BL], BF16)
        nc.sync.dma_start(xTs[:], xT[:])
        wbxS = const.tile([128, 4 * 256], BF16)
        nc.sync.dma_start(wbxS[:], wbx[:])
        wbigS = const.tile([128, 6, 4 * 256], f)
        nc.sync.dma_start(_r(wbigS[:]), _r(wbig.transpose([1, 0, 2])))
        hI = const.tile([128, 64], f)          # h K-form init (h(-1))
        nc.sync.dma_start(_r(hI[:]), _r(hk0[:]))
        STATE = const.tile([128, 6 * 64], f)   # gates f,i,o,g | ct | tc
        nc.sync.dma_start(STATE[:, 4 * 64:5 * 64], c0[:])
        wmkS = const.tile([128, 2, 2, 128], f)
        nc.sync.dma_start(
            _r(wmkS[:]), _r(wmk.rearrange("p (a b c) -> p a b c", a=2, b=2)))
        mempTS = const.tile([128, 2, BL * SP], f)
        nc.sync.dma_start(
            _r(mempTS[:]), _r(mempT.rearrange("p (k n) -> p k n", k=2)))
        maskBs = const.tile([128, NCH * BL], f)
        nc.sync.dma_start(maskBs[:], maskB[:])
        eyeS = const.tile([32, 32], f)
        nc.sync.dma_start(eyeS[:], eye[:])
        mempS = const.tile([128, NCH, UC], f)
        nc.sync.dma_start(_r(mempS[:]), _r(memp.rearrange("p (c u) -> p c u", c=NCH)))
        wattnS = const.tile([128, 4, 2, 128], f)
        nc.sync.dma_start(
            _r(wattnS[:]), _r(wattn.rearrange("p (a b c) -> p a b c", a=4, b=2)))
        if has_blstm:
            blstmS = const.tile([1, 4 * 256], f)
            nc.sync.dma_start(blstmS[:], blstm[:])
            onesS = const.tile([1, BL], f)
            nc.sync.dma_start(onesS[:], onesd[:])
        wfcS = const.tile([128, 2, VL], BF16)
        nc.sync.dma_start(wfcS[:], wfc.transpose([1, 0, 2]))

        # attn K-form history (bf16): the fc lhsT, written by batched attn-proj
        attn_fcT = const.tile([128, 2, AFT], BF16)

        # keysT [128, 2(j), NCH*128]: rows = v in stperm(j) order
        keysT = const.tile([128, 2, BL * SP], f)
        for j in range(2):
            for h_ in range(2):
                kp = ps_fc.tile([128, 512], f, tag="fc")
                for kin in range(2):
                    mm(kp[:], wmkS[:, kin, j, :],
                       mempTS[:, kin, 512 * h_:512 * (h_ + 1)],
                       start=(kin == 0), stop=(kin == 1))
                nc.vector.tensor_copy(_r(keysT[:, j, 512 * h_:512 * (h_ + 1)]),
                                      kp[:])

        def z_x(t, p):
            """Open z(t) quarter-pair p [32, 512] with the bf16 x chunk."""
            ztp = ps_z.tile([32, 512], f, tag=f"zp{p}", name=f"zP{t}_{p}")
            mmf(ztp[:], xTs[:, BL * t:BL * (t + 1)],
                wbxS[:, 512 * p:512 * (p + 1)], start=True, stop=False)
            return ztp

        def z_h(ztp, hsrc, wof, p, close=False):
            """h chunks (fp32r, weight chunks wof..wof+1). close=True ends
            the accumulation here (t=0 has no ctx contribution)."""
            for k in range(2):
                mmz(ztp[:], hsrc[:, 32 * k:32 * (k + 1)],
                    wbigS[:, wof + k, 512 * p:512 * (p + 1)],
                    start=False,
                    stop=(close and not has_blstm and k == 1))
            if close and has_blstm:
                mmf(ztp[:], onesS[:], blstmS[:, 512 * p:512 * (p + 1)],
                    start=False, stop=True)

        def z_open(t, hsrc, wof, close=False):
            zt = [z_x(t, p) for p in range(2)]
            for p in range(2):
                z_h(zt[p], hsrc, wof, p, close)
            return zt

        def z_close(zt, csrc):
            """Close z's accumulation with the ctx chunks (+ bias)."""
            for p in range(2):
                for k in range(2):
                    mmz(zt[p][:], csrc[:, 32 * k:32 * (k + 1)],
                        wbigS[:, 4 + k, 512 * p:512 * (p + 1)],
                        start=False, stop=(not has_blstm and k == 1))
                if has_blstm:
                    mmf(zt[p][:], onesS[:], blstmS[:, 512 * p:512 * (p + 1)],
                        start=False, stop=True)

        def fc_banks(m, blist):
            """fc matmuls for banks blist of 128-row tile m; PSUM->SBUF copies
            ride the idle gpsimd queue so ACT/DVE stay clear for the chain."""
            r0, cc = 128 * m, 32 + 128 * m
            nrow = min(128, MROWS - r0)
            for b_ in blist:
                fp = ps_fc.tile([128, FCB], f, tag="fc")
                for k in range(2):
                    mmf(fp[0:nrow, :], attn_fcT[:, k, cc:cc + nrow],
                        wfcS[:, k, FCB * b_:FCB * (b_ + 1)],
                        start=(k == 0), stop=(k == 1))
                fs = work.tile([128, FCB], BF16, tag="fs", bufs=6)
                if b_ % 2 == 0:
                    nc.scalar.copy(fs[0:nrow, :], fp[0:nrow, :])
                else:
                    nc.vector.tensor_copy(fs[0:nrow, :], fp[0:nrow, :])
                nc.sync.dma_start(out[r0:r0 + nrow, FCB * b_:FCB * (b_ + 1)],
                                  fs[0:nrow, :])

        def fc_sched(t):
            """(m, slotA, slotB, slotC) bank ranges for step t's fc work --
            three PE-gap slots sized to bridge the DVE interlocks."""
            if not do_fc or t < 4:
                return None, [], [], []
            if t < 28:
                m = t // 4 - 1
                b0 = 4 * (t % 4)
                return m, [b0], [b0 + 1], [b0 + 2, b0 + 3]
            m = 6
            b0 = 8 * (t - 28)
            return m, [b0, b0 + 1], [b0 + 2, b0 + 3], [b0 + 4, b0 + 5, b0 + 6, b0 + 7]

        for _rep in range(repeat):
         # preamble z(0): x + original-Wr h chunks, no ctx (attn(0)=0)
         zt = z_open(0, hI, 0, close=True)
         for t in range(T):
            # --- scores PSUM: gpsimd writes the mask bias early (slot was
            # freed by last step's exp; plain write, all scores accumulate) ---
            PS = ps_sc.tile([128, NCH, BL], f, tag="sc")
            nc.vector.tensor_copy(PS.rearrange("p a b -> p (a b)"), maskBs[:])
            # --- gates: tanh(z') -> STATE[:, 0:256], one ACT per quarter ---
            for q in range(4):
                nc.scalar.activation(
                    STATE[32 * q:32 * (q + 1), 0:4 * 64],
                    zt[q // 2][:, 256 * (q % 2):256 * (q % 2 + 1)], AF.Tanh)
            # --- cell: u=(1+tf)*ct, v=(1+ti)*tg ; ct' = 0.5u + v ---
            UVt = work.tile([128, 2 * 64], f, tag="uv")
            nc.vector.scalar_tensor_tensor(
                _ap(UVt[:], 0, [[128, 128], [64, 2], [1, 64]]),
                _ap(STATE[:], 0, [[384, 128], [64, 2], [1, 64]]),
                1.0,
                _ap(STATE[:], 4 * 64, [[384, 128], [-64, 2], [1, 64]]),
                op0=ALU.add, op1=ALU.mult)
            nc.vector.scalar_tensor_tensor(
                STATE[:, 4 * 64:5 * 64], UVt[:, 0:64], 0.5, UVt[:, 64:128],
                op0=ALU.mult, op1=ALU.add)
            nc.scalar.activation(STATE[:, 5 * 64:6 * 64], STATE[:, 4 * 64:5 * 64],
                                 AF.Tanh, scale=0.5)
            HB = work.tile([128, 64], f, tag="hb")
            nc.vector.scalar_tensor_tensor(
                HB[:], STATE[:, 2 * 64:3 * 64], 1.0, STATE[:, 5 * 64:6 * 64],
                op0=ALU.add, op1=ALU.mult)
            # --- h K-form: one DVE 32x32 block transpose + fp32r mirror
            # (f32r StreamTranspose is invalid ISA; fp32r consumers need a
            # rounded-typed producer) ---
            HBT = work.tile([128, 64], f, tag="hbt")
            nc.vector.transpose(HBT[:], HB[:])
            HBTr = work.tile([128, 64], f, tag="hbtr")
            nc.vector.tensor_copy(_r(HBTr[:]), HBT[:])
            # --- scores accumulate onto the bias (fp32r: 2c/row unramped) ---
            for m_ in range(NCH):
                for j in range(2):
                    mm(PS[:, m_, :], keysT[:, j, 128 * m_:128 * (m_ + 1)],
                       HBTr[:, 32 * j:32 * (j + 1)],
                       start=False, stop=(j == 1))
            if 'scores' in amp:
                sps = ps_fc.tile([128, NCH, BL], f, tag="fc")
                for m_ in range(NCH):
                    for j in range(2):
                        mm(sps[:, m_, :], keysT[:, j, 128 * m_:128 * (m_ + 1)],
                           HBTr[:, 32 * j:32 * (j + 1)],
                           start=(j == 0), stop=(j == 1))
            if 'gates' in amp:
                gsc = work.tile([128, 256], f, tag="gsc")
                for q in range(4):
                    nc.scalar.activation(gsc[32 * q:32 * (q + 1), :],
                                         zt[q // 2][:, 256 * (q % 2):256 * (q % 2 + 1)],
                                         AF.Tanh)
            if 'cell' in amp:
                csc = work.tile([128, 128], f, tag="csc")
                nc.vector.scalar_tensor_tensor(
                    csc[:], STATE[:, 0:128], 1.0, STATE[:, 128:256],
                    op0=ALU.add, op1=ALU.mult)
                nc.vector.scalar_tensor_tensor(
                    csc[:, 0:64], STATE[:, 0:64], 0.5, STATE[:, 64:128],
                    op0=ALU.mult, op1=ALU.add)
                nc.vector.transpose(csc[:, 0:64], STATE[:, 0:64])
                nc.vector.tensor_copy(csc[:, 64:128], STATE[:, 0:64])
            if 'zmm' in amp:
                zsc = ps_fc.tile([32, 512], f, tag="fc")
                for k in range(2):
                    mmz(zsc[:], HBTr[:, 32 * k:32 * (k + 1)],
                        wbigS[:, 2 + k, 0:512],
                        start=(k == 0), stop=(k == 1))
                mmz(zsc[:], HBTr[:, 0:32], wbigS[:, 4, 0:512],
                    start=False, stop=False)
                mmz(zsc[:], HBTr[:, 32:64], wbigS[:, 5, 0:512],
                    start=False, stop=True)
            # --- z(t+1) pair-0 x chunk: exactly fills the exp window ---
            if t + 1 < T:
                zt = [None, None]
                zt[0] = z_x(t + 1, 0)
            # --- exp in two halves so ctx c=0..3 start after the first
            # (masked lanes underflow to 0), fp32r-rounded out ---
            ET = work.tile([128, NCH, BL], f, tag="et")
            nc.scalar.activation(_r(ET[:, 0:4, :]), PS[:, 0:4, :], AF.Exp)
            nc.scalar.activation(_r(ET[:, 4:8, :]), PS[:, 4:8, :], AF.Exp)
            # --- context (+denominator) ---
            CX = ps_ca.tile([BL, UC], f, tag="catp")
            for c in range(NCH):
                mm(CX[:], ET[:, c, :], mempS[:, c, :],
                   start=(c == 0), stop=(c == NCH - 1))
            # --- remaining z(t+1) x+h chunks fill the normalize window ---
            if t + 1 < T:
                z_h(zt[0], HBTr, 2, 0)
                zt[1] = z_x(t + 1, 1)
                z_h(zt[1], HBTr, 2, 1)
            rec = work.tile([BL, 1], f, tag="rc")
            nc.vector.reciprocal(rec[:], CX[:, U:U + 1])
            CXS = work.tile([BL, U], f, tag="cxs")
            nc.vector.tensor_scalar_mul(CXS[:], CX[:, 0:U], rec[:])
            # ctx K-form (natural): 2 PE transposes into one tile + 1 copy
            tpc = ps_ca.tile([128, 64], f, tag="catp")
            for k in range(2):
                nc.tensor.matmul(tpc[:, 32 * k:32 * (k + 1)],
                                 CXS[:, 128 * k:128 * (k + 1)], eyeS[:],
                                 is_transpose=True, skip_group_check=True)
            ctxK = work.tile([128, 64], f, tag="ctxK")
            nc.vector.tensor_copy(_r(ctxK[:]), tpc[:])
            if t + 1 < T:
                z_close(zt, ctxK)
            # --- attn proj for fc lhsT (off critical path), K-form direct ---
            if do_attn:
                atp = ps_ca.tile([128, 64], f, tag="catp")
                ach = [HBTr[:, 0:32], HBTr[:, 32:64],
                       ctxK[:, 0:32], ctxK[:, 32:64]]
                for k in range(2):
                    for c in range(4):
                        mm(atp[:, 32 * k:32 * (k + 1)], wattnS[:, c, k, :],
                           ach[c], start=(c == 0), stop=(c == 3))
                nc.vector.tensor_copy(
                    _ap(attn_fcT[:], BL * (t + 1),
                        [[2 * AFT, 128], [AFT, 2], [1, BL]]),
                    _ap(atp[:], 0, [[64, 128], [32, 2], [1, BL]]))
            # --- fc: PE filler during the gates+cell window of t+1 ---
            fm, fa, fb, fcr = fc_sched(t)
            if fm is not None:
                fc_banks(fm, list(fa) + list(fb) + list(fcr))
         # --- tail fc: last 64-row tile ---
         if do_fc:
             fc_banks(7, range(NFCB))

    nc.compile()
    return nc


def _prep(inputs):
    """Host-side prep: shard + fold scales into weights. Returns in_maps list."""
    f = np.float32
    import ml_dtypes
    bf = ml_dtypes.bfloat16
    emb = np.asarray(inputs["emb_table"], f)
    W_k = np.asarray(inputs["W_k"], f)
    W_r = np.asarray(inputs["W_r"], f)
    b_l = np.asarray(inputs["b_lstm"], f)
    W_mem = np.asarray(inputs["W_mem"], f)
    W_attn = np.asarray(inputs["W_attn"], f)
    W_fc = np.asarray(inputs["W_fc"], f)
    idx_in = np.asarray(inputs["inputs"])
    memory = np.asarray(inputs["memory"], f)
    h0 = np.asarray(inputs["sample_h"], f)
    c0_ = np.asarray(inputs["sample_c"], f)

    # stperm[k, p]: u-row held at (chunk k, partition p) of ST-interleaved K-form
    p_ = np.arange(128)
    stperm = np.stack([64 * (p_ // 32) + 32 * k + (p_ % 32) for k in range(2)])

    # column permutation: per u-quarter q: [f_q*.5 | i_q*.5 | o_q*.5 | g_q]
    # jax z-split order: i [0,U), f [U,2U), g [2U,3U), o [3U,4U)
    cols, scl = [], []
    for q in range(4):
        uq = np.arange(64 * q, 64 * (q + 1))
        cols += [U + uq, 0 + uq, 3 * U + uq, 2 * U + uq]
        scl += [np.full(64, .5, f), np.full(64, .5, f),
                np.full(64, .5, f), np.ones(64, f)]
    perm_c = np.concatenate(cols)
    colscale = np.concatenate(scl)

    # attn fold: Wk_a = W_k rows E..E+U (the attn part of cell_in)
    Wk_a = W_k[E:E + U].astype(np.float64)           # [256, 1024]
    Wha = (W_attn[0:U].astype(np.float64) @ Wk_a).astype(f)   # h -> z via attn
    Wca = (W_attn[U:2 * U].astype(np.float64) @ Wk_a).astype(f)  # ctx -> z

    Wc_k = W_k[:, perm_c] * colscale                 # [384, 1024] col-permuted
    Wr_orig = (0.5 * W_r)[:, perm_c] * colscale      # [256, 1024] (t=0)
    Wr_eff = (0.5 * (W_r + Wha))[:, perm_c] * colscale
    Wca_c = Wca[:, perm_c] * colscale
    wbig = np.stack([Wr_orig[stperm[0]], Wr_orig[stperm[1]],
                     Wr_eff[stperm[0]], Wr_eff[stperm[1]],
                     Wca_c[0:128], Wca_c[128:256]])  # [6, 128, 1024]
    wbx = np.ascontiguousarray(Wc_k[0:128]).astype(bf)
    b_p = np.ascontiguousarray((b_l[perm_c] * colscale).reshape(1, 4 * 256))
    has_blstm = bool(np.any(b_p != 0))

    # wattnT[p, c, k, :]: chunks c=0,1 h-part (stperm rows), c=2,3 ctx (natural)
    wattnT = np.zeros((128, 4, 2, 128), f)
    for c in range(2):
        Wh = (0.5 * W_attn)[stperm[c]]        # [128, 256]
        for k in range(2):
            wattnT[:, c, k, :] = Wh[:, 128 * k:128 * (k + 1)]
    for c in range(2):
        Wc = W_attn[U + 128 * c:U + 128 * (c + 1)]
        for k in range(2):
            wattnT[:, 2 + c, k, :] = Wc[:, 128 * k:128 * (k + 1)]
    wattnT = wattnT.reshape(128, 4 * 2 * 128)

    Wm = 0.5 * W_mem                          # [256(u-in), 256(v)]
    wmk = np.zeros((128, 2, 2, 128), f)
    for kin in range(2):
        for j in range(2):
            wmk[:, kin, j, :] = Wm[128 * kin:128 * (kin + 1)][:, stperm[j]]
    wmk = wmk.reshape(128, 2 * 2 * 128)
    eye = np.eye(32, dtype=f)

    # dense mask bias: B[p,(c,b)] = 0 if (b==4c+l(p) and s(p)<S) else -BIGB
    maskB = np.full((4, SP, NCH, BL), -BIGB, f)
    for c in range(NCH):
        for l in range(4):
            maskB[l, :S, c, 4 * c + l] = 0.0
    maskB = maskB.reshape(128, NCH * BL)

    x_emb = emb[idx_in]                      # [B, T, E] host gather
    in_maps = []
    for g in range(NB):
        bs = slice(BL * g, BL * (g + 1))
        xTl = np.ascontiguousarray(
            x_emb[bs].transpose(2, 1, 0).reshape(E, T * BL)).astype(bf)
        # hK0[32q+uu, 32k+b] = 2*h0[b, 64q+32k+uu]
        h2 = 2 * h0[bs]                       # [32, 256]
        hk0 = np.zeros((128, 64), f)
        for k in range(2):
            hk0[:, 32 * k:32 * (k + 1)] = h2[:, stperm[k]].T
        c0d = np.ascontiguousarray(
            (2 * c0_[bs]).reshape(BL, 4, 64).transpose(1, 0, 2).reshape(128, 64))
        mloc = memory[bs]                    # [BL, S, U]
        mp = np.zeros((4, SP, NCH, UC), f)
        for c in range(NCH):
            for blo in range(4):
                mp[blo, :S, c, :U] = mloc[4 * c + blo]
                mp[blo, :S, c, U] = 1.0
        mp = mp.reshape(128, NCH * UC)
        mt = np.zeros((BL, SP, U), f)
        mt[:, :S, :] = mloc
        mt = np.ascontiguousarray(
            mt.reshape(BL * SP, U).T.reshape(2, 128, BL * SP)
            .transpose(1, 0, 2).reshape(128, 2 * BL * SP))
        for v in range(NV):
            m = {"xT": xTl, "wbig": wbig, "wbx": wbx, "wattn": wattnT,
                 "wmk": wmk,
                 "wfc": np.ascontiguousarray(
                     W_fc[:, VL * v:VL * (v + 1)].reshape(2, 128, VL)).astype(bf),
                 "memp": mp, "mempT": mt,
                 "maskB": maskB,
                 "hk0": hk0, "c0": c0d, "eye": eye}
            if has_blstm:
                m["blstm"] = b_p
                m["onesd"] = np.ones((1, BL), f)
            in_maps.append(m)
    return in_maps, has_blstm


_CACHE = {}


def kernel(**inputs) -> np.ndarray:
    in_maps, has_blstm = _prep(inputs)
    if has_blstm not in _CACHE:
        _CACHE[has_blstm] = build_program(has_blstm)
    nc = _CACHE[has_blstm]
    res = bass_utils.run_bass_kernel_spmd(
        nc, in_maps, core_ids=list(range(NB * NV)))
    outs = [r["out"] for r in res.results]   # each [960, 8000] bf16, rows (t, b)
    full = np.empty((B, T, V), np.float32)
    for g in range(NB):
        for v in range(NV):
            o = np.asarray(outs[NV * g + v], dtype=np.float32)
            o = o.reshape(T, BL, VL)
            full[BL * g:BL * (g + 1), :, VL * v:VL * (v + 1)] = o.transpose(1, 0, 2)
    b_fc = np.asarray(inputs["b_fc"], np.float32)
    if np.any(b_fc != 0):
        full = full + b_fc
    return full
